# revision 30
# baseline (speedup 1.0000x reference)
"""Trainium2 Bass kernel for nn_CSLRTransformer (dense transformer, 8 cores).

Sharding: 4 batch elements x 2-way sequence split = 8 cores.
Core c handles batch b=c//2, token half h=c%2 (tokens h*512..h*512+511 "own").
Token order on each core is LOCAL: [own 512 | peer 512]; attention is
key-permutation invariant so no global order is needed until the conv head,
which only needs a 3-position halo handled with per-core edge masks.

Per layer: LN1(full) -> K,V(full)+Q(own) -> attn(own q) -> out-proj(own)
-> residual -> LN2(own) -> FFN(own) -> residual -> pair AllGather of the
own half; peer half reassembled as (slot0+slot1) - own (SPMD-uniform).

The attention path (Q/K/V projections, P*V context, out-proj) runs in
fp8e4m3 with DoubleRow perf mode (256-deep contraction, 2 output cols per
cycle): near-uniform attention dilutes fp8 noise to ~2e-3 at the output.
The undiluted value path (embed, FFN1/2, convs, input poses) stays bf16 -
fp8 there costs ~5% relative error per GEMM, measured 8.6e-2 end-to-end.
fp8 weights are pre-scaled x64 on the host; descales fold into the PSUM
evacuations.  Scores run in bf16 carrying the x64 q/k scale, divided out
inside the softmax exp scale.  LayerNorm rsqrt is computed as
exp(-0.5*ln(var)); a post-compile pass remaps/dedupes activation-table
loads so the whole stack uses one table (no per-layer reloads).  LN
normalize is emitted sign-negated (one fused scalar_tensor_tensor off the
stats psum); consumers' weights are negated on the host.

Engine split: PE matmuls; Act exp/rsqrt/gelu + ff2 evac; DVE all other
PSUM evacuations; Pool (gpsimd) all SBUF-side elementwise (LN normalize,
residuals, peer assembly) since Pool cannot access PSUM.
"""
import numpy as np
import ml_dtypes

import concourse.bacc as bacc
import concourse.bass as bass
import concourse.mybir as mybir
import concourse.tile as tile
from concourse.bass_utils import run_bass_kernel_spmd

dt = mybir.dt
AF = mybir.ActivationFunctionType
ALU = mybir.AluOpType
PM = mybir.MatmulPerfMode

P = 128
B, T, IN_DIM, D, H, NCLS = 4, 1024, 231, 512, 8, 1296
NL, DFF, DH = 8, 2048, 64
TH = T // 2            # 512 own tokens
TP = TH // 2           # 256 own pooled positions
KIN = 256              # padded embed contraction (231 -> 256)
NCP = 1408             # padded classes (1296 -> 11*128)
EPS = 1e-5
F32 = dt.float32
F32R = dt.float32r
BF16 = dt.bfloat16
F8 = dt.float8e4
SW = 64.0              # fp8 weight scale
ISW = 1.0 / SW
ISW2 = 1.0 / (SW * SW)

_CACHE = {}


def _build(single_core=False, fuse_bias=True):
    nc = bacc.Bacc("TRN2", target_bir_lowering=False, debug=False, num_devices=8)

    # ---- DRAM I/O ----
    poses_t = nc.dram_tensor("poses_t", [KIN, T], BF16, kind="ExternalInput")
    pos_t = nc.dram_tensor("pos_t", [D, T], F32, kind="ExternalInput")
    edges = nc.dram_tensor("edges", [P, 2], F32, kind="ExternalInput")
    emb_wt = nc.dram_tensor("emb_wt", [4, P, KIN], BF16, kind="ExternalInput")
    emb_b = nc.dram_tensor("emb_b", [D], F32, kind="ExternalInput")
    ln0_g = nc.dram_tensor("ln0_g", [D], F32, kind="ExternalInput")
    q_wt = nc.dram_tensor("q_wt", [NL, 4, P, D], F8, kind="ExternalInput")
    k_wt = nc.dram_tensor("k_wt", [NL, 4, P, D], F8, kind="ExternalInput")
    v_wt = nc.dram_tensor("v_wt", [NL, P, 2, 2, D], F8, kind="ExternalInput")
    qkv_b = nc.dram_tensor("qkv_b", [NL, 3 * D], F32, kind="ExternalInput")
    out_wt = nc.dram_tensor("out_wt", [NL, 4, P, D], F8, kind="ExternalInput")
    out_b = nc.dram_tensor("out_b", [NL, D], F32, kind="ExternalInput")
    ff1_wt = nc.dram_tensor("ff1_wt", [NL, 16, P, D], BF16, kind="ExternalInput")
    ff1_b = nc.dram_tensor("ff1_b", [NL, DFF], F32, kind="ExternalInput")
    ff2_wt = nc.dram_tensor("ff2_wt", [NL, 4, P, DFF], BF16, kind="ExternalInput")
    ff2_b = nc.dram_tensor("ff2_b", [NL, D], F32, kind="ExternalInput")
    c1_wt = nc.dram_tensor("c1_wt", [5, 4, P, D], BF16, kind="ExternalInput")
    bn1_s = nc.dram_tensor("bn1_s", [D], F32, kind="ExternalInput")
    bn1_t = nc.dram_tensor("bn1_t", [D], F32, kind="ExternalInput")
    c2_wt = nc.dram_tensor("c2_wt", [3, 4, P, D], BF16, kind="ExternalInput")
    bn2_s = nc.dram_tensor("bn2_s", [D], F32, kind="ExternalInput")
    bn2_t = nc.dram_tensor("bn2_t", [D], F32, kind="ExternalInput")
    fc1_wt = nc.dram_tensor("fc1_wt", [2, P, D], F32R, kind="ExternalInput")
    fc1_b = nc.dram_tensor("fc1_b", [D // 2], F32, kind="ExternalInput")
    fc2_wt = nc.dram_tensor("fc2_wt", [11, P, D // 2], F32R, kind="ExternalInput")
    fc2_b = nc.dram_tensor("fc2_b", [NCP], F32, kind="ExternalInput")
    out_d = nc.dram_tensor("out", [NCP, TP], F32, kind="ExternalOutput")

    with tile.TileContext(nc) as tc:
        with (
            tc.tile_pool(name="state", bufs=1) as state,
            tc.tile_pool(name="act1", bufs=1) as act1,
            tc.tile_pool(name="act2", bufs=2) as act2,
            tc.tile_pool(name="wts", bufs=3) as wts,
            tc.tile_pool(name="wlin", bufs=1) as wlin,
            tc.tile_pool(name="wff", bufs=1) as wff,
            tc.tile_pool(name="ps_sc", bufs=3, space="PSUM") as ps_sc,
            tc.tile_pool(name="ps_mm", bufs=2, space="PSUM") as ps_mm,
            tc.tile_pool(name="dram", bufs=3, space="DRAM") as dram,
            nc.allow_low_precision(reason="fp8/bf16 activations by design"),
        ):
            # ---------- constants / persistent ----------
            ones_f32 = state.tile([P, P], F32)
            nc.vector.memset(ones_f32[:], 1.0)
            ones_sq = state.tile([P, P], F32R)
            nc.vector.tensor_copy(ones_sq[:], ones_f32[:])
            ones_row = state.tile([1, DH], F32R)
            nc.vector.tensor_copy(ones_row[:], ones_f32[0:1, 0:DH])
            ones_bf = state.tile([P, P], BF16)
            nc.vector.tensor_copy(ones_bf[:], ones_f32[:])
            zeros_sb = state.tile([P, 512], F32)
            nc.vector.memset(zeros_sb[:], 0.0)
            x_sb = state.tile([P, 4, T], F32R)      # residual stream (local)
            xblk = state.tile([P, 4, T], F32R)      # block-residual save

            def load_pcol(dr, n):
                # [n*128] dram vector -> [128, n] sbuf (d on partitions)
                t_ = state.tile([P, n], F32, tag=f"b{n}_{dr.tensor.name}")
                nc.sync.dma_start(t_[:], dr.rearrange("(o p) -> p o", p=P))
                return t_

            emb_b_sb = load_pcol(emb_b.ap(), 4)
            ln0g_sb = load_pcol(ln0_g.ap(), 4)
            bn1s_sb = load_pcol(bn1_s.ap(), 4)
            bn1t_sb = load_pcol(bn1_t.ap(), 4)
            bn2s_sb = load_pcol(bn2_s.ap(), 4)
            bn2t_sb = load_pcol(bn2_t.ap(), 4)
            fc1b_sb = load_pcol(fc1_b.ap(), 2)
            fc2b_sb = load_pcol(fc2_b.ap(), 11)
            edges_sb = state.tile([P, 2], F32)
            nc.sync.dma_start(edges_sb[:], edges[:])

            # ---------- helpers ----------
            def linear8(x, w_tiles, nkp, nm, ncols, evac, out, nt=0):
                """out[:, mo, nt*512:...] = (w.T @ x-pairs) via DoubleRow.
                w_tiles: list of nm sbuf tiles [P, nkp, 2, P] fp8.
                x: fp8 [P, 2*nkp, >=ncols]."""
                for mo in range(nm):
                    cs = min(512, ncols)
                    ps = ps_mm.tile([P, 512], F32, tag="ps")
                    for kp in range(nkp):
                        nc.tensor.matmul(
                            ps[:, :cs], w_tiles[mo][:, kp],
                            x[:, 2 * kp:2 * kp + 2, nt * 512:nt * 512 + cs],
                            perf_mode=PM.DoubleRow,
                            start=(kp == 0), stop=(kp == nkp - 1))
                    evac(ps[:, :cs], mo, nt, out)
                return out

            def load_w8(w_dram, nkp, nm, pool, tag):
                # all weight DMAs ride SP/HWDGE: gpsimd DMAs occupy the Pool
                # ENGINE (software DGE) which we need for elementwise work
                tiles = []
                for mo in range(nm):
                    t_ = pool.tile([P, nkp, 2, P], F8, tag=f"{tag}{mo}")
                    nc.sync.dma_start(t_[:].rearrange("p a b c -> p (a b c)"),
                                      w_dram[mo])
                    tiles.append(t_)
                return tiles

            def load_w16(w_dram, nk, nm, pool, tag):
                tiles = []
                for mo in range(nm):
                    t_ = pool.tile([P, nk, P], BF16, tag=f"{tag}{mo}")
                    nc.sync.dma_start(t_[:].rearrange("p a b -> p (a b)"),
                                      w_dram[mo])
                    tiles.append(t_)
                return tiles

            def linear16(x, w_tiles, nk, nm, ncols, evac, out, nt=0):
                for mo in range(nm):
                    cs = min(512, ncols)
                    ps = ps_mm.tile([P, 512], F32, tag="ps")
                    for ko in range(nk):
                        nc.tensor.matmul(
                            ps[:, :cs], w_tiles[mo][:, ko],
                            x[:, ko, nt * 512:nt * 512 + cs],
                            start=(ko == 0), stop=(ko == nk - 1))
                    evac(ps[:, :cs], mo, nt, out)
                return out

            def ev_bias_alt(bias_sb):
                # alternate DVE/Act so neither engine serializes the psum
                # rotation during the projection phase
                def _e(ps, mo, nt, out):
                    o = out[:, mo, nt * 512:nt * 512 + ps.shape[-1]]
                    b = bias_sb[:, mo:mo + 1]
                    if (mo + nt) % 2 == 0:
                        nc.vector.tensor_scalar(o, ps, b, None, ALU.add)
                    else:
                        nc.scalar.activation(o, ps, AF.Identity, bias=b)
                return _e

            def ev_relu_alt(bias_sb):
                # r1 = max(ps + 64*b, 0); mostly DVE, every 4th on Act
                def _e(ps, mo, nt, out):
                    o = out[:, mo, nt * 512:nt * 512 + ps.shape[-1]]
                    b = bias_sb[:, mo:mo + 1]
                    if mo % 2 == 1:
                        nc.scalar.activation(o, ps, AF.Relu, bias=b)
                    else:
                        nc.vector.scalar_tensor_tensor(
                            o, ps, b, zeros_sb[:, 0:ps.shape[-1]],
                            ALU.add, ALU.max)
                return _e

            def ev_resid(scale, bias_sb):
                # x_own += ps*scale (+bias); the fused single-op path is used
                # when the host verified the biases are zero
                def _e(ps, mo, nt, out):
                    cs = ps.shape[-1]
                    xsl = x_sb[:, mo, nt * 512:nt * 512 + cs]
                    if fuse_bias:
                        nc.vector.scalar_tensor_tensor(
                            xsl, ps, scale, xsl, ALU.mult, ALU.add)
                    else:
                        y_ = act1.tile([P, 512], F32R, tag="yres")
                        nc.vector.tensor_scalar(
                            y_[:, 0:cs], ps, scale, bias_sb[:, mo:mo + 1],
                            ALU.mult, ALU.add)
                        nc.gpsimd.tensor_tensor(xsl, xsl, y_[:, 0:cs],
                                                ALU.add)
                return _e

            # ---- LayerNorm pieces.  Normalize is emitted NEGATED:
            # t = m - x (one fused scalar_tensor_tensor straight off the
            # stats psum), h = t * r = -(x-m)*r; consumers' weights are
            # negated on the host (or gamma is negated for LN0).
            SLO = slice(0, 512)

            def ln_sq_ko(sq, x, sl, ko, eng):
                if eng == 0:
                    nc.scalar.activation(sq[:, ko, :], x[:, ko, sl],
                                         AF.Square)
                elif eng == 1:
                    nc.vector.tensor_tensor(sq[:, ko, :], x[:, ko, sl],
                                            x[:, ko, sl], ALU.mult)
                else:
                    nc.gpsimd.tensor_tensor(sq[:, ko, :], x[:, ko, sl],
                                            x[:, ko, sl], ALU.mult)

            def ln_stats1_ko(pss, x, sl, ko):
                nc.tensor.matmul(pss[:, 0, :], ones_sq[:], x[:, ko, sl],
                                 start=(ko == 0), stop=(ko == 3))

            def ln_stats2_ko(pss, sq, ko):
                nc.tensor.matmul(pss[:, 1, :], ones_bf[:], sq[:, ko, :],
                                 start=(ko == 0), stop=(ko == 3))

            def ln_finish(pss, x, sl, out, gamma=None):
                ts = []
                for ko in range(4):
                    t_ = act1.tile([P, 512], F32, tag=f"nrm{ko}")
                    nc.vector.scalar_tensor_tensor(t_[:], pss[:, 0, :],
                                                   1.0 / D, x[:, ko, sl],
                                                   ALU.mult, ALU.subtract)
                    ts.append(t_)
                msq = act1.tile([P, 512], F32, tag="msq")
                nc.scalar.activation(msq[:], pss[:, 0, :], AF.Square,
                                     scale=1.0 / D)
                va = act1.tile([P, 512], F32, tag="va")
                nc.vector.tensor_scalar(va[:], pss[:, 1, :], 1.0 / D, EPS,
                                        ALU.mult, ALU.add)
                nc.gpsimd.tensor_tensor(va[:], va[:], msq[:], ALU.subtract)
                lnv = act1.tile([P, 512], F32, tag="lnv")
                nc.scalar.activation(lnv[:], va[:], AF.Ln)
                r = act1.tile([P, 512], F32, tag="r")
                nc.scalar.activation(r[:], lnv[:], AF.Exp, scale=-0.5)
                for ko in range(4):
                    eng = nc.vector if ko % 2 == 0 else nc.gpsimd
                    if gamma is not None:
                        t2 = act1.tile([P, 512], F32, tag="lnt2")
                        eng.tensor_tensor(t2[:], ts[ko][:], r[:], ALU.mult)
                        eng.tensor_scalar(out[:, ko, sl], t2[:],
                                          gamma[:, ko:ko + 1], None, ALU.mult)
                    else:
                        eng.tensor_tensor(out[:, ko, sl], ts[ko][:], r[:],
                                          ALU.mult)

            def ln_chain(x, sl, out, sq_engs=(0, 1, 2, 1), gamma=None):
                pss = ps_sc.tile([P, 2, 512], F32, tag="ps_sc")
                sq = act1.tile([P, 4, 512], BF16, tag="sq")
                for ko in range(4):
                    ln_sq_ko(sq, x, sl, ko, sq_engs[ko])
                for ko in range(4):
                    ln_stats1_ko(pss, x, sl, ko)
                for ko in range(4):
                    ln_stats2_ko(pss, sq, ko)
                ln_finish(pss, x, sl, out, gamma=gamma)

            def linear8_resid_ln(src, w_tiles, nkp, scale, bias_sb, ln_out,
                                 extra_ko=None, do_ln=True,
                                 sq_engs=(0, 1, 2, 1), bf16=False):
                """4-mo linear with fused residual into x_sb own half;
                the next LayerNorm's stats matmuls + squares are interleaved
                per-ko one step behind the evacuations."""
                ev = ev_resid(scale, bias_sb)
                if do_ln:
                    pss = ps_sc.tile([P, 2, 512], F32, tag="ps_sc")
                    sq = act1.tile([P, 4, 512], BF16, tag="sq")

                def emit_ko(ko):
                    if extra_ko is not None:
                        extra_ko(ko)
                    if do_ln:
                        ln_stats1_ko(pss, x_sb, SLO, ko)
                        ln_sq_ko(sq, x_sb, SLO, ko, sq_engs[ko])

                for mo in range(4):
                    ps = ps_mm.tile([P, 512], F32, tag="ps")
                    if bf16:
                        for ko in range(nkp):
                            nc.tensor.matmul(
                                ps[:], w_tiles[mo][:, ko],
                                src[:, ko, 0:512],
                                start=(ko == 0), stop=(ko == nkp - 1))
                    else:
                        for kp in range(nkp):
                            nc.tensor.matmul(
                                ps[:], w_tiles[mo][:, kp],
                                src[:, 2 * kp:2 * kp + 2, 0:512],
                                perf_mode=PM.DoubleRow,
                                start=(kp == 0), stop=(kp == nkp - 1))
                    ev(ps[:], mo, 0, None)
                    if mo >= 1:
                        emit_ko(mo - 1)
                emit_ko(3)
                if do_ln:
                    for ko in range(4):
                        ln_stats2_ko(pss, sq, ko)
                    ln_finish(pss, x_sb, SLO, ln_out)

            # ---------- embed + LN0 + pos ----------
            poses_sb = act1.tile([P, 2, T], BF16, tag="poses")
            for ko in range(2):
                nc.sync.dma_start(poses_sb[:, ko, :],
                                  poses_t[ko * P:(ko + 1) * P, :])
            emb_tiles = load_w16(emb_wt, 2, 4, wts, "emb")
            xe = act1.tile([P, 4, T], F32R, tag="r1")
            for nt in range(2):
                linear16(poses_sb, emb_tiles, 2, 4, T - nt * 512,
                         ev_bias_alt(emb_b_sb), xe, nt=nt)
            # xe carries x64; LN0 self-normalizes the scale away.
            # LN0 writes straight into x_sb (ln0_g host-negated), then pos
            # adds in-place (halves).
            ln_chain(xe, slice(0, 512), x_sb, gamma=ln0g_sb)
            ln_chain(xe, slice(512, 1024), x_sb, gamma=ln0g_sb)
            for half in range(2):
                pos_sb = act1.tile([P, 4, TH], F32, tag=f"s{half}")
                for ko in range(4):
                    nc.sync.dma_start(
                        pos_sb[:, ko, :],
                        pos_t[ko * P:(ko + 1) * P,
                              half * TH:(half + 1) * TH])
                for ko in range(4):
                    eng = nc.vector if ko % 2 == 0 else nc.gpsimd
                    eng.tensor_tensor(
                        x_sb[:, ko, half * TH:(half + 1) * TH],
                        x_sb[:, ko, half * TH:(half + 1) * TH],
                        pos_sb[:, ko, :], ALU.add)

            # ---------- transformer layers ----------
            # Software pipeline: layer li's LN1(own) chain runs in layer
            # li-1's tail (own-half x is final before the gather), so Q
            # starts immediately at each layer top.  The peer half is
            # reconstructed from the AllGather at the top, its LN chain
            # hidden under Q/K/V-own matmuls.
            pending = [None, None]

            v_ext = state.tile([P, 4, 2, H, 66], F8)
            nc.gpsimd.memset(v_ext[:, :, :, :, 64:66], 0.0)
            nc.gpsimd.memset(v_ext[:, :, :, :, 64:65], 1.0)

            qkvb_l = [load_pcol(qkv_b[li], 12) for li in range(NL)]
            outb_l = [load_pcol(out_b[li], 4) for li in range(NL)]
            ff1b_l = [load_pcol(ff1_b[li], 16) for li in range(NL)]
            ff2b_l = [load_pcol(ff2_b[li], 4) for li in range(NL)]

            def make_assembly(li_prev):
                b_out = pending[1]
                if li_prev == 7:
                    def _asm():
                        s0 = act1.tile([P, 4, 12], F32R, tag="s0")
                        s1 = act1.tile([P, 4, 12], F32R, tag="s1")
                        nc.sync.dma_start(
                            s0[:], b_out[0].rearrange("(ko p) t -> p ko t", p=P))
                        nc.sync.dma_start(
                            s1[:], b_out[1].rearrange("(ko p) t -> p ko t", p=P))
                        nc.gpsimd.tensor_tensor(s0[:], s0[:], s1[:], ALU.add)
                        nc.gpsimd.tensor_tensor(
                            x_sb[:, :, 512:518], s0[:, :, 0:6],
                            x_sb[:, :, 0:6], ALU.subtract)
                        nc.gpsimd.tensor_tensor(
                            x_sb[:, :, 1018:1024], s0[:, :, 6:12],
                            x_sb[:, :, 506:512], ALU.subtract)
                    return _asm

                def _asm():
                    s0 = act1.tile([P, 4, TH], BF16, tag="s0")
                    s1 = act1.tile([P, 4, TH], BF16, tag="s1")
                    for ko in range(4):
                        nc.sync.dma_start(s0[:, ko, :],
                                          b_out[0, ko * P:(ko + 1) * P, :])
                        nc.sync.dma_start(s1[:, ko, :],
                                          b_out[1, ko * P:(ko + 1) * P, :])
                    for ko in range(4):
                        eng = nc.vector if ko % 2 == 0 else nc.gpsimd
                        ssum = act1.tile([P, 512], F32, tag=f"nrm{ko}")
                        eng.tensor_tensor(ssum[:], s0[:, ko, :],
                                          s1[:, ko, :], ALU.add)
                        eng.tensor_tensor(x_sb[:, ko, TH:T],
                                          ssum[:],
                                          x_sb[:, ko, 0:TH],
                                          ALU.subtract)
                return _asm

            # LN1(own) for layer 0 (x own is final after LN0+pos)
            h1 = act1.tile([P, 4, T], F8, tag="h1")
            ln_chain(x_sb, slice(0, 512), h1)
            lw = dict(
                q=load_w8(q_wt[0], 2, 4, wlin, "q"),
                k=load_w8(k_wt[0], 2, 4, wlin, "k"),
                o=load_w8(out_wt[0], 2, 4, wlin, "o"))
            wv0 = wlin.tile([P, 2, 2, D], F8, tag="wv")
            nc.sync.dma_start(
                wv0[:].rearrange("p a b c -> p (a b c)"),
                v_wt[0].rearrange("p a b c -> p (a b c)"))
            lw['v'] = wv0

            for li in range(NL):
                qkvb_sb = qkvb_l[li]
                q_tiles, k_tiles, o_tiles, wv = (lw['q'], lw['k'], lw['o'],
                                                 lw['v'])

                # ---- peer reconstruction (DMAs fire first) ----
                if pending[0] is not None:
                    pending[0]()
                    pending[0] = None

                # ---- own-token projections (h1 own ready from tail) ----
                q_t = act1.tile([P, 4, TH], BF16, tag="qt")
                linear8(h1, q_tiles, 2, 4, TH,
                        ev_bias_alt(qkvb_sb[:, 0:]), q_t)
                k_t = act1.tile([P, 4, T], BF16, tag="kt")
                linear8(h1, k_tiles, 2, 4, T,
                        ev_bias_alt(qkvb_sb[:, 4:]), k_t, nt=0)

                def v_tiles(tts):
                    for tt in tts:
                        ps = ps_mm.tile([P, 512], F32, tag="ps")
                        for kp in range(2):
                            nc.tensor.matmul(
                                ps[:],
                                h1[:, 2 * kp:2 * kp + 2,
                                   tt * P:(tt + 1) * P],
                                wv[:, kp],
                                perf_mode=PM.DoubleRow,
                                start=(kp == 0), stop=(kp == 1))
                        dst = v_ext[:, tt // 2, tt % 2, :, 0:64]
                        srcv = ps[:].rearrange("p (h d) -> p h d", d=64)
                        if tt % 2 == 0:
                            nc.vector.tensor_copy(dst, srcv)
                        else:
                            nc.scalar.activation(dst, srcv, AF.Identity)

                v_tiles(range(4))

                # ---- attention ----
                # exp arg: scores carry x64 q * x64 k -> /4096, then /8
                esc = 0.125 * ISW2
                ctx = act1.tile([P, 4, TH], F8, tag="ctx")
                p_ts = []
                for _pi in range(2):
                    p_t_buf = act2.tile([P, 4, 2, 2, TH], F8, tag="pt")
                    p_ts.append(p_t_buf)

                def emit_ctx(mo, hhs=(0, 1)):
                    p_t = p_ts[mo % 2]
                    for hh in hhs:
                        h = 2 * mo + hh
                        bp = hh * 64
                        pscr = ps_sc.tile([P, 2, 512], F32, tag="ps_sc")
                        for tp in range(4):
                            nc.tensor.matmul(
                                pscr[0:65, 0, :], v_ext[:, tp, :, h, 0:65],
                                p_t[:, tp, :, hh, :],
                                perf_mode=PM.DoubleRow,
                                start=(tp == 0), stop=(tp == 3))
                        rcp = act1.tile([1, TH], F32R, tag="rcp")
                        nc.vector.reciprocal(rcp[:], pscr[64:65, 0, :])
                        nc.tensor.matmul(pscr[0:64, 1, :], ones_row[:],
                                         rcp[:], start=True, stop=True)
                        rcpb = act1.tile([64, TH], F32, tag=f"rcpb{hh}")
                        if hh == 0:
                            nc.vector.tensor_copy(rcpb[:], pscr[0:64, 1, :])
                        else:
                            nc.scalar.activation(rcpb[:], pscr[0:64, 1, :],
                                                 AF.Identity)
                        nc.vector.tensor_tensor(
                            ctx[bp:bp + 64, mo, :], pscr[0:64, 0, :],
                            rcpb[:], ALU.mult)

                def sc_kt(mo, kt):
                    p_t = p_ts[mo % 2]
                    psb = ps_sc.tile([P, 2, 512], F32, tag="ps_sc")
                    nc.tensor.matmul(
                        psb[:, 0, :], k_t[0:64, mo, kt * P:(kt + 1) * P],
                        q_t[0:64, mo, :], start=True, stop=True)
                    nc.tensor.matmul(
                        psb[:, 1, :], k_t[64:128, mo, kt * P:(kt + 1) * P],
                        q_t[64:128, mo, :], start=True, stop=True)
                    nc.scalar.activation(
                        p_t[:, kt // 2, kt % 2, :, :], psb[:],
                        AF.Exp, scale=esc)

                # own-key scores first: Act starts exp while the peer half
                # (LN1-peer, K/V-peer below) is still being produced
                for mo in range(4):
                    for kt in range(4):
                        sc_kt(mo, kt)

                # ---- peer half: LN1 + K,V (overlaps own-key exps) ----
                ln_chain(x_sb, slice(512, 1024), h1, sq_engs=(1, 2, 1, 2))
                linear8(h1, k_tiles, 2, 4, T,
                        ev_bias_alt(qkvb_sb[:, 4:]), k_t, nt=1)
                v_tiles(range(4, 8))
                ff1_tiles = load_w16(ff1_wt[li], 4, 16, wff, "f1")
                ff2_tiles = load_w16(ff2_wt[li], 16, 4, wff, "f2")

                # peer-key scores + ctx interleave
                prev = None
                for mo in range(4):
                    for kt in range(4, 8):
                        sc_kt(mo, kt)
                        if kt == 5 and prev is not None:
                            emit_ctx(prev, hhs=(0,))
                        if kt == 7 and prev is not None:
                            emit_ctx(prev, hhs=(1,))
                            prev = None
                    prev = mo
                emit_ctx(3)

                # preload next layer's projection weights during attention
                if li < NL - 1:
                    lw = dict(
                        q=load_w8(q_wt[li + 1], 2, 4, wlin, "q"),
                        k=load_w8(k_wt[li + 1], 2, 4, wlin, "k"),
                        o=load_w8(out_wt[li + 1], 2, 4, wlin, "o"))
                    wv2 = wlin.tile([P, 2, 2, D], F8, tag="wv")
                    nc.sync.dma_start(
                        wv2[:].rearrange("p a b c -> p (a b c)"),
                        v_wt[li + 1].rearrange("p a b c -> p (a b c)"))
                    lw['v'] = wv2

                # out-proj + fused residual, LN2 stats interleaved
                h2 = act1.tile([P, 4, TH], BF16, tag="h2")
                linear8_resid_ln(ctx, o_tiles, 2, ISW2, outb_l[li], h2)

                # FFN (bf16: undiluted value path needs > fp8 precision)
                r1 = act1.tile([P, 16, TH], BF16, tag="r1")
                linear16(h2, ff1_tiles, 4, 16, TH, ev_relu_alt(ff1b_l[li]),
                         r1)

                # ff2 + fused residual; tail interleaves block residual,
                # per-ko gather DMA, and the NEXT layer's LN1(own)
                if li < 7:
                    b_in = dram.tile([D, TH], BF16, tag="agin")
                    b_out = dram.tile([2, D, TH], BF16, tag="agout")
                    b_in_r = b_in.rearrange("(ko p) t -> p ko t", p=P)
                else:
                    b_in = dram.tile([D, 12], F32R, tag="agin7")
                    b_out = dram.tile([2, D, 12], F32R, tag="agout7")
                    b_in_r = None

                xg = act1.tile([P, 4, TH], BF16, tag="xg")

                def tail_extra(ko):
                    eng = nc.vector if ko % 2 == 0 else nc.gpsimd
                    if li in (3, 5, 7):
                        eng.tensor_tensor(x_sb[:, ko, 0:TH],
                                          x_sb[:, ko, 0:TH],
                                          xblk[:, ko, 0:TH], ALU.add)
                    if li in (1, 3, 5):
                        eng.tensor_copy(xblk[:, ko, 0:TH],
                                        x_sb[:, ko, 0:TH])
                    if li < 7:
                        eng.tensor_copy(xg[:, ko, :], x_sb[:, ko, 0:TH])
                        nc.sync.dma_start(b_in_r[:, ko, :], xg[:, ko, :])

                if li < 7:
                    h1 = act1.tile([P, 4, T], F8, tag="h1")
                    linear8_resid_ln(r1, ff2_tiles, 16, 1.0, ff2b_l[li],
                                     h1, extra_ko=tail_extra, bf16=True)
                else:
                    linear8_resid_ln(r1, ff2_tiles, 16, 1.0, ff2b_l[li],
                                     None, extra_ko=tail_extra, do_ln=False,
                                     bf16=True)
                    bi = b_in.rearrange("(ko p) t -> p ko t", p=P)
                    nc.sync.dma_start(bi[:, :, 0:6], x_sb[:, :, 0:6])
                    nc.sync.dma_start(bi[:, :, 6:12], x_sb[:, :, 506:512])

                if single_core:
                    # timing-only stand-in for TimelineSim (no collectives)
                    nc.sync.dma_start(b_out[0], b_in[:])
                    nc.sync.dma_start(b_out[1], b_in[:])
                else:
                    nc.gpsimd.collective_compute(
                        "AllGather", ALU.bypass,
                        ins=[b_in.opt()], outs=[b_out.opt()],
                        replica_groups=[[0, 1], [2, 3], [4, 5], [6, 7]])
                pending[1] = b_out
                pending[0] = make_assembly(li)

            # ---------- head: pool -> conv1 -> conv2 -> fc1 -> fc2 ----------
            pending[0]()
            pending[0] = None
            # avg-pool(2): the 0.5 factor is folded into conv1 weights
            xp = act1.tile([P, 4, TH], BF16, tag="h2")
            nc.gpsimd.tensor_tensor(xp[:], x_sb[:, :, 0:T:2],
                                    x_sb[:, :, 1:T:2], ALU.add)
            # padded conv input [128, 4, 262] = [L(3) | own 256 | R(3)]
            xpe = act1.tile([P, 4, 262], BF16, tag="ysb")
            nc.gpsimd.tensor_copy(xpe[:, :, 3:259], xp[:, :, 0:TP])
            nc.gpsimd.tensor_scalar(xpe[:, :, 0:3], xp[:, :, 509:512],
                                    edges_sb[:, 0:1], None, ALU.mult)
            nc.gpsimd.tensor_scalar(xpe[:, :, 259:262], xp[:, :, 256:259],
                                    edges_sb[:, 1:2], None, ALU.mult)

            def conv_block(src, ntaps, w_dram, ncols, bn_s, bn_t, out):
                # w_dram: [ntaps, 4(mo), 128, 512] fp8 mo-blocked per tap
                for mo in range(4):
                    ps = ps_mm.tile([P, 512], F32, tag="ps")
                    for k in range(ntaps):
                        wt = wts.tile([P, 4, P], BF16, tag="cw")
                        nc.sync.dma_start(
                            wt[:].rearrange("p a b -> p (a b)"),
                            w_dram[k, mo])
                        for ko in range(4):
                            nc.tensor.matmul(
                                ps[:, 0:ncols], wt[:, ko],
                                src[:, ko, k:k + ncols],
                                start=(k == 0 and ko == 0),
                                stop=(k == ntaps - 1 and ko == 3))
                    nc.scalar.activation(out[:, mo, :], ps[:, 0:ncols],
                                         AF.Gelu, bias=bn_t[:, mo:mo + 1],
                                         scale=bn_s[:, mo:mo + 1])

            y1e = act1.tile([P, 4, 258], BF16, tag="r1")
            conv_block(xpe, 5, c1_wt, 258, bn1s_sb, bn1t_sb, y1e)
            # conv2 zero-pads its input at the GLOBAL sequence edges: kill the
            # computed y1 halo column on the outer side of each boundary core
            nc.gpsimd.tensor_scalar(y1e[:, :, 0:1], y1e[:, :, 0:1],
                                    edges_sb[:, 0:1], None, ALU.mult)
            nc.gpsimd.tensor_scalar(y1e[:, :, 257:258], y1e[:, :, 257:258],
                                    edges_sb[:, 1:2], None, ALU.mult)
            y2c = act1.tile([P, 4, TP], F32R, tag="h2")
            conv_block(y1e, 3, c2_wt, TP, bn2s_sb, bn2t_sb, y2c)
            # fc1 (512->256) + gelu  (fp32r for head accuracy)
            hfc = act1.tile([P, 2, TP], F32R, tag="qt")
            for mo in range(2):
                ps = ps_mm.tile([P, 512], F32, tag="ps")
                wt = wts.tile([P, 4 * P], F32R, tag="wmo")
                nc.sync.dma_start(wt[:], fc1_wt[mo])
                for ko in range(4):
                    nc.tensor.matmul(ps[:, 0:TP], wt[:, ko * P:(ko + 1) * P],
                                     y2c[:, ko, :],
                                     start=(ko == 0), stop=(ko == 3))
                nc.scalar.activation(hfc[:, mo, :], ps[:, 0:TP], AF.Gelu,
                                     bias=fc1b_sb[:, mo:mo + 1])
            # fc2 (256->1408 padded)
            ologit = act1.tile([P, 11, TP], F32, tag="r1")
            for mo in range(11):
                ps = ps_mm.tile([P, 512], F32, tag="ps")
                wt = wts.tile([P, 2 * P], F32R, tag="wmo")
                nc.sync.dma_start(wt[:], fc2_wt[mo])
                for ko in range(2):
                    nc.tensor.matmul(ps[:, 0:TP], wt[:, ko * P:(ko + 1) * P],
                                     hfc[:, ko, :],
                                     start=(ko == 0), stop=(ko == 1))
                nc.scalar.activation(ologit[:, mo, :], ps[:, 0:TP],
                                     AF.Identity, bias=fc2b_sb[:, mo:mo + 1])
            for mo in range(11):
                nc.sync.dma_start(out_d[mo * P:(mo + 1) * P, :],
                                  ologit[:, mo, :])

    nc.compile()
    _fix_act_tables(nc)
    return nc


def _fix_act_tables(nc):
    """Bass assigns each activation function its FIRST containing table from
    act_info.json (Ln -> natural_log, Exp -> exp_and_others), which makes
    alternating Ln/Exp reload tables every time (~4.5us each).  Rewrite: a
    table load whose following segment is servable by the still-loaded table
    is dropped; otherwise, if servable by natural_log_exp_and_others (which
    holds exp+ln+identity+relu+square), it is remapped there.  Walrus adopts
    pre-placed loads, and every activation's function remains inside the
    loaded set, so hardware semantics are unchanged."""
    from concourse.bacc import get_activation_tables
    tabs = list(get_activation_tables(nc.m.arch).items())
    target = next(i for i, (name, _) in enumerate(tabs)
                  if name == 'natural_log_exp_and_others')
    tfuncs = tabs[target][1]
    for blk in nc.m.functions[0].blocks:
        insts = blk.instructions
        # segment funcs following each load (until the next load)
        seg_funcs = {}
        cur_load = None
        for inst in insts:
            if isinstance(inst, mybir.InstLoadActFuncSet):
                cur_load = id(inst)
                seg_funcs[cur_load] = set()
            elif isinstance(inst, mybir.InstActivation) and cur_load is not None:
                seg_funcs[cur_load].add(inst.func)
        keep = []
        live = None                     # funcs of the currently loaded table
        for inst in insts:
            if isinstance(inst, mybir.InstLoadActFuncSet):
                seg = seg_funcs[id(inst)]
                if live is not None and seg <= live:
                    continue            # already servable: drop the load
                if seg <= tfuncs:
                    inst.act_func_set_id = target
                    live = tfuncs
                else:
                    live = tabs[inst.act_func_set_id][1]
            keep.append(inst)
        insts[:] = keep


def _q8(w):
    return np.clip(w, -448, 448).astype(ml_dtypes.float8_e4m3)


def _moblk8(w_t, nk, nm):
    """[nk*128, nm*128] fp32 -> [nm, 128, nk*128] fp8 DoubleRow pairs:
    out[mo, p, (kp,i,m)] = w_t[(2*kp+i)*128+p, mo*128+m] * SW"""
    a = _q8(w_t * SW).reshape(nk // 2, 2, P, nm, P)
    a = a.transpose(3, 2, 0, 1, 4)          # [nm, p, kp, i, m]
    return np.ascontiguousarray(a.reshape(nm, P, nk * P))


def _prep_inputs(inputs):
    """Host-side: transposes, padding, LN-affine folding, fp8 quantization,
    per-core shards."""
    f = lambda k: np.asarray(inputs[k], dtype=np.float32)
    poses = f('poses')
    embed_w, embed_b = f('embed_w'), f('embed_b')
    ln0_g, ln0_b = f('ln0_g'), f('ln0_b')
    inw, inb = f('inw'), f('inb')
    outw, outb = f('outw'), f('outb')
    ln1g, ln1b = f('ln1g'), f('ln1b')
    ln2g, ln2b = f('ln2g'), f('ln2b')
    ff1w, ff1b = f('ff1w'), f('ff1b')
    ff2w, ff2b = f('ff2w'), f('ff2b')
    conv1w, conv1b = f('conv1w'), f('conv1b')
    bn1g, bn1b, bn1m, bn1v = f('bn1g'), f('bn1b'), f('bn1m'), f('bn1v')
    conv2w, conv2b = f('conv2w'), f('conv2b')
    bn2g, bn2b, bn2m, bn2v = f('bn2g'), f('bn2b'), f('bn2m'), f('bn2v')
    fc1w, fc1b = f('fc1w'), f('fc1b')
    fc2w, fc2b = f('fc2w'), f('fc2b')

    def moblk(w_t, nk, nm):
        # fp32 [nk*128, nm*128] -> [nm, 128, nk*128] (plain, for head fcs)
        return np.ascontiguousarray(
            w_t.reshape(nk, P, nm, P).transpose(2, 1, 0, 3).reshape(nm, P, nk * P))

    shared = {}
    ewt = np.zeros((KIN, D), np.float32)
    ewt[:IN_DIM] = embed_w.T
    shared['emb_wt'] = moblk(ewt, 2, 4).astype(ml_dtypes.bfloat16)
    shared['emb_b'] = embed_b
    shared['ln0_g'] = -ln0_g

    qkv_wt = np.empty((NL, D, 3 * D), np.float32)
    qkv_bf = np.empty((NL, 3 * D), np.float32)
    out_wtf = np.empty((NL, D, D), np.float32)
    out_bf = np.empty((NL, D), np.float32)
    ff1_wtf = np.empty((NL, D, DFF), np.float32)
    ff1_bf = np.empty((NL, DFF), np.float32)
    ff2_wtf = np.empty((NL, DFF, D), np.float32)
    ff2_bf = np.empty((NL, D), np.float32)
    for l in range(NL):
        w = inw[l]                      # [3D, D]
        qkv_wt[l] = (w * ln1g[l][None, :]).T
        qkv_bf[l] = inb[l] + w @ ln1b[l]
        out_wtf[l] = outw[l].T
        bias_v = qkv_bf[l, 2 * D:]
        out_bf[l] = outb[l] + outw[l] @ bias_v
        ff1_wtf[l] = (ff1w[l] * ln2g[l][None, :]).T
        ff1_bf[l] = ff1b[l] + ff1w[l] @ ln2b[l]
        ff2_wtf[l] = ff2w[l].T
        ff2_bf[l] = ff2b[l]
    if 'fuse_flag' not in _CACHE:
        _CACHE['fuse_flag'] = bool(
            np.allclose(out_bf, 0.0) and np.allclose(ff2_bf, 0.0))
    qkv_bs = qkv_bf.copy()
    qkv_bs[:, :2 * D] *= SW             # q,k biases ride the x64 scale
    shared.update(qkv_b=qkv_bs, out_b=out_bf, ff1_b=ff1_bf,
                  ff2_b=ff2_bf)
    # LN outputs are emitted negated on-chip: fold the sign into every
    # weight that consumes an LN output (q,k,v,ff1) and into ln0_g
    shared['q_wt'] = np.stack([_moblk8(-qkv_wt[l][:, 0:D], 4, 4)
                               for l in range(NL)])
    shared['k_wt'] = np.stack([_moblk8(-qkv_wt[l][:, D:2 * D], 4, 4)
                               for l in range(NL)])
    # V weights in DoubleRow rhs layout [NL, p, kp, i, 512]
    vt = qkv_wt[:, :, 2 * D:].reshape(NL, 2, 2, P, D)   # [l, kp, i, p, v]
    shared['v_wt'] = np.ascontiguousarray(
        _q8(vt.transpose(0, 3, 1, 2, 4) * -SW))
    shared['out_wt'] = np.stack([_moblk8(out_wtf[l], 4, 4)
                                 for l in range(NL)])
    shared['ff1_wt'] = np.stack([moblk(-ff1_wtf[l], 4, 16)
                                 for l in range(NL)]).astype(ml_dtypes.bfloat16)
    shared['ff2_wt'] = np.stack([moblk(ff2_wtf[l], 16, 4)
                                 for l in range(NL)]).astype(ml_dtypes.bfloat16)

    bn1sc = bn1g / np.sqrt(bn1v + EPS)
    bn2sc = bn2g / np.sqrt(bn2v + EPS)
    c1t = conv1w.transpose(2, 1, 0) * 0.5           # [5, D_in, D_out]
    shared['c1_wt'] = np.stack([moblk(c1t[k], 4, 4)
                                for k in range(5)]).astype(ml_dtypes.bfloat16)
    shared['bn1_s'] = bn1sc
    shared['bn1_t'] = (conv1b - bn1m) * bn1sc + bn1b
    c2t = conv2w.transpose(2, 1, 0)
    shared['c2_wt'] = np.stack([moblk(c2t[k], 4, 4)
                                for k in range(3)]).astype(ml_dtypes.bfloat16)
    shared['bn2_s'] = bn2sc
    shared['bn2_t'] = (conv2b - bn2m) * bn2sc + bn2b
    shared['fc1_wt'] = moblk(np.ascontiguousarray(fc1w.T), 4, 2)
    shared['fc1_b'] = fc1b
    f2 = np.zeros((D // 2, NCP), np.float32)
    f2[:, :NCLS] = fc2w.T
    shared['fc2_wt'] = moblk(f2, 2, 11)
    f2b = np.zeros((NCP,), np.float32)
    f2b[:NCLS] = fc2b
    shared['fc2_b'] = f2b

    inv = 1.0 / (10000.0 ** (np.arange(0, D, 2, dtype=np.float32) / D))
    si = np.arange(T, dtype=np.float32)[:, None] * inv[None, :]
    pos = np.stack([np.sin(si), np.cos(si)], -1).reshape(T, D)
    pos = pos.astype(np.float32)
    pos_t_g = (pos + ln0_b[None, :]).T.copy()       # [D, T]

    in_maps = []
    for c in range(8):
        b, h = c // 2, c % 2
        own = slice(h * TH, (h + 1) * TH)
        peer = slice((1 - h) * TH, (2 - h) * TH)
        p_loc = np.concatenate([poses[b, own], poses[b, peer]], 0)
        pt = np.zeros((KIN, T), np.float32)
        pt[:IN_DIM] = p_loc.T
        pos_loc = np.concatenate([pos_t_g[:, own], pos_t_g[:, peer]], 1)
        edges_a = np.zeros((P, 2), np.float32)
        edges_a[:, 0] = 1.0 if h == 1 else 0.0
        edges_a[:, 1] = 1.0 if h == 0 else 0.0
        m = dict(shared)
        m['poses_t'] = pt.astype(ml_dtypes.bfloat16)
        m['pos_t'] = pos_loc
        m['edges'] = edges_a
        in_maps.append({k: np.ascontiguousarray(v) for k, v in m.items()})
    return in_maps


def _get_runner():
    """Build the module once and cache a jitted SPMD executable whose weight
    operands stay device-resident between calls."""
    if 'runner' in _CACHE:
        return _CACHE['runner']
    fuse = _CACHE.get('fuse_flag', True)
    import jax
    import concourse.mybir as mybir_
    from concourse import bass2jax
    from jax.experimental.shard_map import shard_map
    from jax.sharding import Mesh, NamedSharding, PartitionSpec

    nc = _build(fuse_bias=fuse)
    bass2jax.install_neuronx_cc_hook()
    partition_name = (nc.partition_id_tensor.name
                      if nc.partition_id_tensor else None)
    in_names, out_names, out_avals, zero_outs = [], [], [], []
    for alloc in nc.m.functions[0].allocations:
        if not isinstance(alloc, mybir_.MemoryLocationSet):
            continue
        name = alloc.memorylocations[0].name
        if alloc.kind == "ExternalInput":
            if name != partition_name:
                in_names.append(name)
        elif alloc.kind == "ExternalOutput":
            shape = tuple(alloc.tensor_shape)
            dtype = mybir_.dt.np(alloc.dtype)
            out_names.append(name)
            out_avals.append(jax.core.ShapedArray(shape, dtype))
            zero_outs.append((shape, dtype))
    n_params = len(in_names)
    all_names = in_names + out_names
    if partition_name is not None:
        all_names.append(partition_name)
    donate = tuple(range(n_params, n_params + len(out_names)))

    def _body(*args):
        operands = list(args)
        if partition_name is not None:
            operands.append(bass2jax.partition_id_tensor())
        outs = bass2jax._bass_exec_p.bind(
            *operands,
            out_avals=tuple(out_avals),
            in_names=tuple(all_names),
            out_names=tuple(out_names),
            lowering_input_output_aliases=(),
            sim_require_finite=True,
            sim_require_nnan=True,
            nc=nc,
        )
        return tuple(outs)

    devices = jax.devices()[:8]
    mesh = Mesh(np.asarray(devices), ("core",))
    spec = PartitionSpec("core")
    sharding = NamedSharding(mesh, spec)
    jitted = jax.jit(
        shard_map(_body, mesh=mesh, in_specs=(spec,) * (n_params + len(out_names)),
                  out_specs=(spec,) * len(out_names), check_rep=False),
        donate_argnums=donate, keep_unused=True)

    runner = dict(jitted=jitted, in_names=in_names, out_names=out_names,
                  zero_outs=zero_outs, sharding=sharding)
    _CACHE['runner'] = runner
    return runner


def _put_args(in_maps):
    import jax
    r = _get_runner()
    args = []
    for name in r['in_names']:
        concat = np.concatenate([in_maps[c][name] for c in range(8)], axis=0)
        args.append(jax.device_put(concat, r['sharding']))
    return args


def _exec(args):
    """Run with device-resident input args; returns per-core result dicts.
    Output (donated) buffers are freshly zero-allocated per call."""
    import jax
    r = _get_runner()
    outs_in = [jax.device_put(np.zeros((8 * s[0],) + s[1:], d), r['sharding'])
               for s, d in r['zero_outs']]
    outs = r['jitted'](*args, *outs_in)
    outs = [np.asarray(o) for o in outs]
    return [{name: outs[i].reshape(8, *r['zero_outs'][i][0])[c]
             for i, name in enumerate(r['out_names'])}
            for c in range(8)]


def _run(in_maps):
    return _exec(_put_args(in_maps))


def kernel(**inputs):
    in_maps = _prep_inputs(inputs)
    results = _run(in_maps)
    out = np.empty((B, T // 2, NCLS), np.float32)
    for c in range(8):
        b, h = c // 2, c % 2
        out[b, h * TP:(h + 1) * TP, :] = results[c]['out'][:NCLS].T
    return out


# revision 31
# speedup vs baseline: 1.0299x; 1.0299x over previous
"""Trainium2 Bass kernel for nn_CSLRTransformer (dense transformer, 8 cores).

Sharding: 4 batch elements x 2-way sequence split = 8 cores.
Core c handles batch b=c//2, token half h=c%2 (tokens h*512..h*512+511 "own").
Token order on each core is LOCAL: [own 512 | peer 512]; attention is
key-permutation invariant so no global order is needed until the conv head,
which only needs a 3-position halo handled with per-core edge masks.

Per layer: LN1(full) -> K,V(full)+Q(own) -> attn(own q) -> out-proj(own)
-> residual -> LN2(own) -> FFN(own) -> residual -> pair AllGather of the
own half; peer half reassembled as (slot0+slot1) - own (SPMD-uniform).

The attention path (Q/K/V projections, P*V context, out-proj) runs in
fp8e4m3 with DoubleRow perf mode (256-deep contraction, 2 output cols per
cycle): near-uniform attention dilutes fp8 noise to ~2e-3 at the output.
The undiluted value path (embed, FFN1/2, convs, input poses) stays bf16 -
fp8 there costs ~5% relative error per GEMM, measured 8.6e-2 end-to-end.
fp8 weights are pre-scaled x64 on the host; descales fold into the PSUM
evacuations.  Scores run in bf16 carrying the x64 q/k scale, divided out
inside the softmax exp scale.  LayerNorm rsqrt is computed as
exp(-0.5*ln(var)); a post-compile pass remaps/dedupes activation-table
loads so the whole stack uses one table (no per-layer reloads).  LN
normalize is emitted sign-negated (one fused scalar_tensor_tensor off the
stats psum); consumers' weights are negated on the host.

Engine split: PE matmuls; Act exp/rsqrt/gelu + ff2 evac; DVE all other
PSUM evacuations; Pool (gpsimd) all SBUF-side elementwise (LN normalize,
residuals, peer assembly) since Pool cannot access PSUM.
"""
import numpy as np
import ml_dtypes

import concourse.bacc as bacc
import concourse.bass as bass
import concourse.mybir as mybir
import concourse.tile as tile
from concourse.bass_utils import run_bass_kernel_spmd

dt = mybir.dt
AF = mybir.ActivationFunctionType
ALU = mybir.AluOpType
PM = mybir.MatmulPerfMode

P = 128
B, T, IN_DIM, D, H, NCLS = 4, 1024, 231, 512, 8, 1296
NL, DFF, DH = 8, 2048, 64
TH = T // 2            # 512 own tokens
TP = TH // 2           # 256 own pooled positions
KIN = 256              # padded embed contraction (231 -> 256)
NCP = 1408             # padded classes (1296 -> 11*128)
EPS = 1e-5
F32 = dt.float32
F32R = dt.float32r
BF16 = dt.bfloat16
F8 = dt.float8e4
SW = 64.0              # fp8 weight scale
ISW = 1.0 / SW
ISW2 = 1.0 / (SW * SW)

_CACHE = {}


def _build(single_core=False, fuse_bias=True):
    nc = bacc.Bacc("TRN2", target_bir_lowering=False, debug=False, num_devices=8)

    # ---- DRAM I/O ----
    poses_t = nc.dram_tensor("poses_t", [KIN, T], BF16, kind="ExternalInput")
    pos_t = nc.dram_tensor("pos_t", [D, T], F32, kind="ExternalInput")
    edges = nc.dram_tensor("edges", [P, 2], F32, kind="ExternalInput")
    emb_wt = nc.dram_tensor("emb_wt", [4, P, KIN], BF16, kind="ExternalInput")
    emb_b = nc.dram_tensor("emb_b", [D], F32, kind="ExternalInput")
    ln0_g = nc.dram_tensor("ln0_g", [D], F32, kind="ExternalInput")
    q_wt = nc.dram_tensor("q_wt", [NL, 4, P, D], F8, kind="ExternalInput")
    k_wt = nc.dram_tensor("k_wt", [NL, 4, P, D], F8, kind="ExternalInput")
    v_wt = nc.dram_tensor("v_wt", [NL, P, 2, 2, D], F8, kind="ExternalInput")
    qkv_b = nc.dram_tensor("qkv_b", [NL, 3 * D], F32, kind="ExternalInput")
    out_wt = nc.dram_tensor("out_wt", [NL, 4, P, D], F8, kind="ExternalInput")
    out_b = nc.dram_tensor("out_b", [NL, D], F32, kind="ExternalInput")
    ff1_wt = nc.dram_tensor("ff1_wt", [NL, 16, P, D], BF16, kind="ExternalInput")
    ff1_b = nc.dram_tensor("ff1_b", [NL, DFF], F32, kind="ExternalInput")
    ff2_wt = nc.dram_tensor("ff2_wt", [NL, 4, P, DFF], BF16, kind="ExternalInput")
    ff2_b = nc.dram_tensor("ff2_b", [NL, D], F32, kind="ExternalInput")
    c1_wt = nc.dram_tensor("c1_wt", [5, 4, P, D], BF16, kind="ExternalInput")
    bn1_s = nc.dram_tensor("bn1_s", [D], F32, kind="ExternalInput")
    bn1_t = nc.dram_tensor("bn1_t", [D], F32, kind="ExternalInput")
    c2_wt = nc.dram_tensor("c2_wt", [3, 4, P, D], BF16, kind="ExternalInput")
    bn2_s = nc.dram_tensor("bn2_s", [D], F32, kind="ExternalInput")
    bn2_t = nc.dram_tensor("bn2_t", [D], F32, kind="ExternalInput")
    fc1_wt = nc.dram_tensor("fc1_wt", [2, P, D], F32R, kind="ExternalInput")
    fc1_b = nc.dram_tensor("fc1_b", [D // 2], F32, kind="ExternalInput")
    fc2_wt = nc.dram_tensor("fc2_wt", [11, P, D // 2], F32R, kind="ExternalInput")
    fc2_b = nc.dram_tensor("fc2_b", [NCP], F32, kind="ExternalInput")
    out_d = nc.dram_tensor("out", [NCP, TP], F32, kind="ExternalOutput")

    with tile.TileContext(nc) as tc:
        with (
            tc.tile_pool(name="state", bufs=1) as state,
            tc.tile_pool(name="act1", bufs=1) as act1,
            tc.tile_pool(name="act2", bufs=2) as act2,
            tc.tile_pool(name="wts", bufs=3) as wts,
            tc.tile_pool(name="wlin", bufs=1) as wlin,
            tc.tile_pool(name="wff", bufs=1) as wff,
            tc.tile_pool(name="ps_sc", bufs=3, space="PSUM") as ps_sc,
            tc.tile_pool(name="ps_mm", bufs=2, space="PSUM") as ps_mm,
            tc.tile_pool(name="dram", bufs=3, space="DRAM") as dram,
            nc.allow_low_precision(reason="fp8/bf16 activations by design"),
        ):
            # ---------- constants / persistent ----------
            ones_f32 = state.tile([P, P], F32)
            nc.vector.memset(ones_f32[:], 1.0)
            ones_sq = state.tile([P, P], F32R)
            nc.vector.tensor_copy(ones_sq[:], ones_f32[:])
            ones_row = state.tile([1, DH], F32R)
            nc.vector.tensor_copy(ones_row[:], ones_f32[0:1, 0:DH])
            ones_bf = state.tile([P, P], BF16)
            nc.vector.tensor_copy(ones_bf[:], ones_f32[:])
            zeros_sb = state.tile([P, 512], F32)
            nc.vector.memset(zeros_sb[:], 0.0)
            x_sb = state.tile([P, 4, T], F32R)      # residual stream (local)
            xblk = state.tile([P, 4, T], F32R)      # block-residual save

            def load_pcol(dr, n):
                # [n*128] dram vector -> [128, n] sbuf (d on partitions)
                t_ = state.tile([P, n], F32, tag=f"b{n}_{dr.tensor.name}")
                nc.sync.dma_start(t_[:], dr.rearrange("(o p) -> p o", p=P))
                return t_

            emb_b_sb = load_pcol(emb_b.ap(), 4)
            ln0g_sb = load_pcol(ln0_g.ap(), 4)
            bn1s_sb = load_pcol(bn1_s.ap(), 4)
            bn1t_sb = load_pcol(bn1_t.ap(), 4)
            bn2s_sb = load_pcol(bn2_s.ap(), 4)
            bn2t_sb = load_pcol(bn2_t.ap(), 4)
            fc1b_sb = load_pcol(fc1_b.ap(), 2)
            fc2b_sb = load_pcol(fc2_b.ap(), 11)
            edges_sb = state.tile([P, 2], F32)
            nc.sync.dma_start(edges_sb[:], edges[:])

            # ---------- helpers ----------
            def linear8(x, w_tiles, nkp, nm, ncols, evac, out, nt=0):
                """out[:, mo, nt*512:...] = (w.T @ x-pairs) via DoubleRow.
                w_tiles: list of nm sbuf tiles [P, nkp, 2, P] fp8.
                x: fp8 [P, 2*nkp, >=ncols]."""
                for mo in range(nm):
                    cs = min(512, ncols)
                    ps = ps_mm.tile([P, 512], F32, tag="ps")
                    for kp in range(nkp):
                        nc.tensor.matmul(
                            ps[:, :cs], w_tiles[mo][:, kp],
                            x[:, 2 * kp:2 * kp + 2, nt * 512:nt * 512 + cs],
                            perf_mode=PM.DoubleRow,
                            start=(kp == 0), stop=(kp == nkp - 1))
                    evac(ps[:, :cs], mo, nt, out)
                return out

            def load_w8(w_dram, nkp, nm, pool, tag):
                # all weight DMAs ride SP/HWDGE: gpsimd DMAs occupy the Pool
                # ENGINE (software DGE) which we need for elementwise work
                tiles = []
                for mo in range(nm):
                    t_ = pool.tile([P, nkp, 2, P], F8, tag=f"{tag}{mo}")
                    nc.sync.dma_start(t_[:].rearrange("p a b c -> p (a b c)"),
                                      w_dram[mo])
                    tiles.append(t_)
                return tiles

            def load_w16(w_dram, nk, nm, pool, tag):
                tiles = []
                for mo in range(nm):
                    t_ = pool.tile([P, nk, P], BF16, tag=f"{tag}{mo}")
                    nc.sync.dma_start(t_[:].rearrange("p a b -> p (a b)"),
                                      w_dram[mo])
                    tiles.append(t_)
                return tiles

            def linear16(x, w_tiles, nk, nm, ncols, evac, out, nt=0):
                for mo in range(nm):
                    cs = min(512, ncols)
                    ps = ps_mm.tile([P, 512], F32, tag="ps")
                    for ko in range(nk):
                        nc.tensor.matmul(
                            ps[:, :cs], w_tiles[mo][:, ko],
                            x[:, ko, nt * 512:nt * 512 + cs],
                            start=(ko == 0), stop=(ko == nk - 1))
                    evac(ps[:, :cs], mo, nt, out)
                return out

            def ev_bias_alt(bias_sb):
                # alternate DVE/Act so neither engine serializes the psum
                # rotation during the projection phase
                def _e(ps, mo, nt, out):
                    o = out[:, mo, nt * 512:nt * 512 + ps.shape[-1]]
                    b = bias_sb[:, mo:mo + 1]
                    if (mo + nt) % 2 == 0:
                        nc.vector.tensor_scalar(o, ps, b, None, ALU.add)
                    else:
                        nc.scalar.activation(o, ps, AF.Identity, bias=b)
                return _e

            def ev_relu_alt(bias_sb):
                # r1 = max(ps + 64*b, 0); mostly DVE, every 4th on Act
                def _e(ps, mo, nt, out):
                    o = out[:, mo, nt * 512:nt * 512 + ps.shape[-1]]
                    b = bias_sb[:, mo:mo + 1]
                    if mo % 2 == 1:
                        nc.scalar.activation(o, ps, AF.Relu, bias=b)
                    else:
                        nc.vector.scalar_tensor_tensor(
                            o, ps, b, zeros_sb[:, 0:ps.shape[-1]],
                            ALU.add, ALU.max)
                return _e

            def ev_resid(scale, bias_sb):
                # x_own += ps*scale (+bias); the fused single-op path is used
                # when the host verified the biases are zero
                def _e(ps, mo, nt, out):
                    cs = ps.shape[-1]
                    xsl = x_sb[:, mo, nt * 512:nt * 512 + cs]
                    if fuse_bias:
                        nc.vector.scalar_tensor_tensor(
                            xsl, ps, scale, xsl, ALU.mult, ALU.add)
                    else:
                        y_ = act1.tile([P, 512], F32R, tag="yres")
                        nc.vector.tensor_scalar(
                            y_[:, 0:cs], ps, scale, bias_sb[:, mo:mo + 1],
                            ALU.mult, ALU.add)
                        nc.gpsimd.tensor_tensor(xsl, xsl, y_[:, 0:cs],
                                                ALU.add)
                return _e

            # ---- LayerNorm pieces.  Normalize is emitted NEGATED:
            # t = m - x (one fused scalar_tensor_tensor straight off the
            # stats psum), h = t * r = -(x-m)*r; consumers' weights are
            # negated on the host (or gamma is negated for LN0).
            SLO = slice(0, 512)

            def ln_sq_ko(sq, x, sl, ko, eng):
                if eng == 0:
                    nc.scalar.activation(sq[:, ko, :], x[:, ko, sl],
                                         AF.Square)
                elif eng == 1:
                    nc.vector.tensor_tensor(sq[:, ko, :], x[:, ko, sl],
                                            x[:, ko, sl], ALU.mult)
                else:
                    nc.gpsimd.tensor_tensor(sq[:, ko, :], x[:, ko, sl],
                                            x[:, ko, sl], ALU.mult)

            def ln_stats1_ko(pss, x, sl, ko):
                nc.tensor.matmul(pss[:, 0, :], ones_sq[:], x[:, ko, sl],
                                 start=(ko == 0), stop=(ko == 3))

            def ln_stats2_ko(pss, sq, ko):
                nc.tensor.matmul(pss[:, 1, :], ones_bf[:], sq[:, ko, :],
                                 start=(ko == 0), stop=(ko == 3))

            def ln_finish(pss, x, sl, out, gamma=None):
                ts = []
                for ko in range(4):
                    t_ = act1.tile([P, 512], F32, tag=f"nrm{ko}")
                    nc.vector.scalar_tensor_tensor(t_[:], pss[:, 0, :],
                                                   1.0 / D, x[:, ko, sl],
                                                   ALU.mult, ALU.subtract)
                    ts.append(t_)
                msq = act1.tile([P, 512], F32, tag="msq")
                nc.scalar.activation(msq[:], pss[:, 0, :], AF.Square,
                                     scale=1.0 / D)
                va = act1.tile([P, 512], F32, tag="va")
                nc.vector.tensor_scalar(va[:], pss[:, 1, :], 1.0 / D, EPS,
                                        ALU.mult, ALU.add)
                nc.gpsimd.tensor_tensor(va[:], va[:], msq[:], ALU.subtract)
                lnv = act1.tile([P, 512], F32, tag="lnv")
                nc.scalar.activation(lnv[:], va[:], AF.Ln)
                r = act1.tile([P, 512], F32, tag="r")
                nc.scalar.activation(r[:], lnv[:], AF.Exp, scale=-0.5)
                for ko in range(4):
                    eng = nc.vector if ko % 2 == 0 else nc.gpsimd
                    if gamma is not None:
                        t2 = act1.tile([P, 512], F32, tag="lnt2")
                        eng.tensor_tensor(t2[:], ts[ko][:], r[:], ALU.mult)
                        eng.tensor_scalar(out[:, ko, sl], t2[:],
                                          gamma[:, ko:ko + 1], None, ALU.mult)
                    else:
                        eng.tensor_tensor(out[:, ko, sl], ts[ko][:], r[:],
                                          ALU.mult)

            def ln_chain(x, sl, out, sq_engs=(0, 1, 2, 1), gamma=None):
                pss = ps_sc.tile([P, 2, 512], F32, tag="ps_sc")
                sq = act1.tile([P, 4, 512], BF16, tag="sq")
                for ko in range(4):
                    ln_sq_ko(sq, x, sl, ko, sq_engs[ko])
                for ko in range(4):
                    ln_stats1_ko(pss, x, sl, ko)
                for ko in range(4):
                    ln_stats2_ko(pss, sq, ko)
                ln_finish(pss, x, sl, out, gamma=gamma)

            def linear8_resid_ln(src, w_tiles, nkp, scale, bias_sb, ln_out,
                                 extra_ko=None, do_ln=True,
                                 sq_engs=(0, 1, 2, 1), bf16=False):
                """4-mo linear with fused residual into x_sb own half;
                the next LayerNorm's stats matmuls + squares are interleaved
                per-ko one step behind the evacuations."""
                ev = ev_resid(scale, bias_sb)
                if do_ln:
                    pss = ps_sc.tile([P, 2, 512], F32, tag="ps_sc")
                    sq = act1.tile([P, 4, 512], BF16, tag="sq")

                def emit_ko(ko):
                    if extra_ko is not None:
                        extra_ko(ko)
                    if do_ln:
                        ln_stats1_ko(pss, x_sb, SLO, ko)
                        ln_sq_ko(sq, x_sb, SLO, ko, sq_engs[ko])

                for mo in range(4):
                    ps = ps_mm.tile([P, 512], F32, tag="ps")
                    if bf16:
                        for ko in range(nkp):
                            nc.tensor.matmul(
                                ps[:], w_tiles[mo][:, ko],
                                src[:, ko, 0:512],
                                start=(ko == 0), stop=(ko == nkp - 1))
                    else:
                        for kp in range(nkp):
                            nc.tensor.matmul(
                                ps[:], w_tiles[mo][:, kp],
                                src[:, 2 * kp:2 * kp + 2, 0:512],
                                perf_mode=PM.DoubleRow,
                                start=(kp == 0), stop=(kp == nkp - 1))
                    ev(ps[:], mo, 0, None)
                    if mo >= 1:
                        emit_ko(mo - 1)
                emit_ko(3)
                if do_ln:
                    for ko in range(4):
                        ln_stats2_ko(pss, sq, ko)
                    ln_finish(pss, x_sb, SLO, ln_out)

            # ---------- embed + LN0 + pos ----------
            poses_sb = act1.tile([P, 2, T], BF16, tag="poses")
            for ko in range(2):
                nc.sync.dma_start(poses_sb[:, ko, :],
                                  poses_t[ko * P:(ko + 1) * P, :])
            emb_tiles = load_w16(emb_wt, 2, 4, wts, "emb")
            xe = act1.tile([P, 4, T], F32R, tag="r1")
            for nt in range(2):
                linear16(poses_sb, emb_tiles, 2, 4, T - nt * 512,
                         ev_bias_alt(emb_b_sb), xe, nt=nt)
            # xe carries x64; LN0 self-normalizes the scale away.
            # LN0 writes straight into x_sb (ln0_g host-negated), then pos
            # adds in-place (halves).
            ln_chain(xe, slice(0, 512), x_sb, gamma=ln0g_sb)
            ln_chain(xe, slice(512, 1024), x_sb, gamma=ln0g_sb)
            for half in range(2):
                pos_sb = act1.tile([P, 4, TH], F32, tag=f"s{half}")
                for ko in range(4):
                    nc.sync.dma_start(
                        pos_sb[:, ko, :],
                        pos_t[ko * P:(ko + 1) * P,
                              half * TH:(half + 1) * TH])
                for ko in range(4):
                    eng = nc.vector if ko % 2 == 0 else nc.gpsimd
                    eng.tensor_tensor(
                        x_sb[:, ko, half * TH:(half + 1) * TH],
                        x_sb[:, ko, half * TH:(half + 1) * TH],
                        pos_sb[:, ko, :], ALU.add)

            # ---------- transformer layers ----------
            # Software pipeline: layer li's LN1(own) chain runs in layer
            # li-1's tail (own-half x is final before the gather), so Q
            # starts immediately at each layer top.  The peer half is
            # reconstructed from the AllGather at the top, its LN chain
            # hidden under Q/K/V-own matmuls.
            pending = [None, None]

            v_ext = state.tile([P, 4, 2, H, 66], F8)
            nc.gpsimd.memset(v_ext[:, :, :, :, 64:66], 0.0)
            nc.gpsimd.memset(v_ext[:, :, :, :, 64:65], 1.0)

            qkvb_l = [load_pcol(qkv_b[li], 12) for li in range(NL)]
            outb_l = [load_pcol(out_b[li], 4) for li in range(NL)]
            ff1b_l = [load_pcol(ff1_b[li], 16) for li in range(NL)]
            ff2b_l = [load_pcol(ff2_b[li], 4) for li in range(NL)]

            def make_assembly(li_prev):
                b_out = pending[1]
                if li_prev == 7:
                    def _asm():
                        s0 = act1.tile([P, 4, 12], F32R, tag="s0")
                        s1 = act1.tile([P, 4, 12], F32R, tag="s1")
                        nc.sync.dma_start(
                            s0[:], b_out[0].rearrange("(ko p) t -> p ko t", p=P))
                        nc.sync.dma_start(
                            s1[:], b_out[1].rearrange("(ko p) t -> p ko t", p=P))
                        nc.gpsimd.tensor_tensor(s0[:], s0[:], s1[:], ALU.add)
                        nc.gpsimd.tensor_tensor(
                            x_sb[:, :, 512:518], s0[:, :, 0:6],
                            x_sb[:, :, 0:6], ALU.subtract)
                        nc.gpsimd.tensor_tensor(
                            x_sb[:, :, 1018:1024], s0[:, :, 6:12],
                            x_sb[:, :, 506:512], ALU.subtract)
                    return _asm

                def _asm():
                    s0 = act1.tile([P, 4, TH], BF16, tag="s0")
                    s1 = act1.tile([P, 4, TH], BF16, tag="s1")
                    for ko in range(4):
                        nc.sync.dma_start(s0[:, ko, :],
                                          b_out[0, ko * P:(ko + 1) * P, :])
                        nc.sync.dma_start(s1[:, ko, :],
                                          b_out[1, ko * P:(ko + 1) * P, :])
                    for ko in range(4):
                        eng = nc.vector if ko % 2 == 0 else nc.gpsimd
                        ssum = act1.tile([P, 512], F32, tag=f"nrm{ko}")
                        eng.tensor_tensor(ssum[:], s0[:, ko, :],
                                          s1[:, ko, :], ALU.add)
                        eng.tensor_tensor(x_sb[:, ko, TH:T],
                                          ssum[:],
                                          x_sb[:, ko, 0:TH],
                                          ALU.subtract)
                return _asm

            # LN1(own) for layer 0 (x own is final after LN0+pos)
            h1 = act1.tile([P, 4, T], F8, tag="h1")
            ln_chain(x_sb, slice(0, 512), h1)
            lw = dict(
                q=load_w8(q_wt[0], 2, 4, wlin, "q"),
                k=load_w8(k_wt[0], 2, 4, wlin, "k"),
                o=load_w8(out_wt[0], 2, 4, wlin, "o"))
            wv0 = wlin.tile([P, 2, 2, D], F8, tag="wv")
            nc.sync.dma_start(
                wv0[:].rearrange("p a b c -> p (a b c)"),
                v_wt[0].rearrange("p a b c -> p (a b c)"))
            lw['v'] = wv0

            for li in range(NL):
                qkvb_sb = qkvb_l[li]
                q_tiles, k_tiles, o_tiles, wv = (lw['q'], lw['k'], lw['o'],
                                                 lw['v'])

                # ---- peer reconstruction (DMAs fire first) ----
                if pending[0] is not None:
                    pending[0]()
                    pending[0] = None

                # ---- own-token projections (h1 own ready from tail) ----
                q_t = act1.tile([P, 4, TH], BF16, tag="qt")
                linear8(h1, q_tiles, 2, 4, TH,
                        ev_bias_alt(qkvb_sb[:, 0:]), q_t)
                k_t = act1.tile([P, 4, T], BF16, tag="kt")
                linear8(h1, k_tiles, 2, 4, T,
                        ev_bias_alt(qkvb_sb[:, 4:]), k_t, nt=0)

                def v_tiles(tts):
                    for tt in tts:
                        ps = ps_mm.tile([P, 512], F32, tag="ps")
                        for kp in range(2):
                            nc.tensor.matmul(
                                ps[:],
                                h1[:, 2 * kp:2 * kp + 2,
                                   tt * P:(tt + 1) * P],
                                wv[:, kp],
                                perf_mode=PM.DoubleRow,
                                start=(kp == 0), stop=(kp == 1))
                        dst = v_ext[:, tt // 2, tt % 2, :, 0:64]
                        srcv = ps[:].rearrange("p (h d) -> p h d", d=64)
                        if tt % 2 == 0:
                            nc.vector.tensor_copy(dst, srcv)
                        else:
                            nc.scalar.activation(dst, srcv, AF.Identity)

                v_tiles(range(4))

                # ---- peer half: LN1 + K,V ----
                ln_chain(x_sb, slice(512, 1024), h1, sq_engs=(1, 2, 1, 2))
                linear8(h1, k_tiles, 2, 4, T,
                        ev_bias_alt(qkvb_sb[:, 4:]), k_t, nt=1)
                v_tiles(range(4, 8))

                # stream FFN weights during attention
                ff1_tiles = load_w16(ff1_wt[li], 4, 16, wff, "f1")
                ff2_tiles = load_w16(ff2_wt[li], 16, 4, wff, "f2")

                # ---- attention ----
                # exp arg: scores carry x64 q * x64 k -> /4096, then /8
                esc = 0.125 * ISW2
                ctx = act1.tile([P, 4, TH], F8, tag="ctx")
                p_ts = []
                for _pi in range(2):
                    p_t_buf = act2.tile([P, 4, 2, 2, TH], F8, tag="pt")
                    p_ts.append(p_t_buf)

                def emit_ctx(mo, hhs=(0, 1)):
                    p_t = p_ts[mo % 2]
                    for hh in hhs:
                        h = 2 * mo + hh
                        bp = hh * 64
                        pscr = ps_sc.tile([P, 2, 512], F32, tag="ps_sc")
                        for tp in range(4):
                            nc.tensor.matmul(
                                pscr[0:65, 0, :], v_ext[:, tp, :, h, 0:65],
                                p_t[:, tp, :, hh, :],
                                perf_mode=PM.DoubleRow,
                                start=(tp == 0), stop=(tp == 3))
                        rcp = act1.tile([1, TH], F32R, tag="rcp")
                        nc.vector.reciprocal(rcp[:], pscr[64:65, 0, :])
                        nc.tensor.matmul(pscr[0:64, 1, :], ones_row[:],
                                         rcp[:], start=True, stop=True)
                        rcpb = act1.tile([64, TH], F32, tag=f"rcpb{hh}")
                        if hh == 0:
                            nc.vector.tensor_copy(rcpb[:], pscr[0:64, 1, :])
                        else:
                            nc.scalar.activation(rcpb[:], pscr[0:64, 1, :],
                                                 AF.Identity)
                        nc.vector.tensor_tensor(
                            ctx[bp:bp + 64, mo, :], pscr[0:64, 0, :],
                            rcpb[:], ALU.mult)

                prev = None
                for mo in range(4):
                    p_t = p_ts[mo % 2]
                    for kt in range(8):
                        psb = ps_sc.tile([P, 2, 512], F32, tag="ps_sc")
                        nc.tensor.matmul(
                            psb[:, 0, :], k_t[0:64, mo, kt * P:(kt + 1) * P],
                            q_t[0:64, mo, :], start=True, stop=True)
                        nc.tensor.matmul(
                            psb[:, 1, :], k_t[64:128, mo, kt * P:(kt + 1) * P],
                            q_t[64:128, mo, :], start=True, stop=True)
                        nc.scalar.activation(
                            p_t[:, kt // 2, kt % 2, :, :], psb[:],
                            AF.Exp, scale=esc)
                        if kt == 1 and prev is not None:
                            emit_ctx(prev, hhs=(0,))
                        if kt == 3 and prev is not None:
                            emit_ctx(prev, hhs=(1,))
                            prev = None
                    prev = mo
                emit_ctx(3)

                # preload next layer's projection weights during attention
                if li < NL - 1:
                    lw = dict(
                        q=load_w8(q_wt[li + 1], 2, 4, wlin, "q"),
                        k=load_w8(k_wt[li + 1], 2, 4, wlin, "k"),
                        o=load_w8(out_wt[li + 1], 2, 4, wlin, "o"))
                    wv2 = wlin.tile([P, 2, 2, D], F8, tag="wv")
                    nc.sync.dma_start(
                        wv2[:].rearrange("p a b c -> p (a b c)"),
                        v_wt[li + 1].rearrange("p a b c -> p (a b c)"))
                    lw['v'] = wv2

                # out-proj + fused residual, LN2 stats interleaved
                h2 = act1.tile([P, 4, TH], BF16, tag="h2")
                linear8_resid_ln(ctx, o_tiles, 2, ISW2, outb_l[li], h2)

                # FFN (bf16: undiluted value path needs > fp8 precision)
                r1 = act1.tile([P, 16, TH], BF16, tag="r1")
                linear16(h2, ff1_tiles, 4, 16, TH, ev_relu_alt(ff1b_l[li]),
                         r1)

                # ff2 + fused residual; tail interleaves block residual,
                # per-ko gather DMA, and the NEXT layer's LN1(own)
                if li < 7:
                    b_in = dram.tile([D, TH], BF16, tag="agin")
                    b_out = dram.tile([2, D, TH], BF16, tag="agout")
                    b_in_r = b_in.rearrange("(ko p) t -> p ko t", p=P)
                else:
                    b_in = dram.tile([D, 12], F32R, tag="agin7")
                    b_out = dram.tile([2, D, 12], F32R, tag="agout7")
                    b_in_r = None

                xg = act1.tile([P, 4, TH], BF16, tag="xg")

                def tail_extra(ko):
                    eng = nc.vector if ko % 2 == 0 else nc.gpsimd
                    if li in (3, 5, 7):
                        eng.tensor_tensor(x_sb[:, ko, 0:TH],
                                          x_sb[:, ko, 0:TH],
                                          xblk[:, ko, 0:TH], ALU.add)
                    if li in (1, 3, 5):
                        eng.tensor_copy(xblk[:, ko, 0:TH],
                                        x_sb[:, ko, 0:TH])
                    if li < 7:
                        eng.tensor_copy(xg[:, ko, :], x_sb[:, ko, 0:TH])
                        nc.sync.dma_start(b_in_r[:, ko, :], xg[:, ko, :])

                if li < 7:
                    h1 = act1.tile([P, 4, T], F8, tag="h1")
                    linear8_resid_ln(r1, ff2_tiles, 16, 1.0, ff2b_l[li],
                                     h1, extra_ko=tail_extra, bf16=True)
                else:
                    linear8_resid_ln(r1, ff2_tiles, 16, 1.0, ff2b_l[li],
                                     None, extra_ko=tail_extra, do_ln=False,
                                     bf16=True)
                    bi = b_in.rearrange("(ko p) t -> p ko t", p=P)
                    nc.sync.dma_start(bi[:, :, 0:6], x_sb[:, :, 0:6])
                    nc.sync.dma_start(bi[:, :, 6:12], x_sb[:, :, 506:512])

                if single_core:
                    # timing-only stand-in for TimelineSim (no collectives)
                    nc.sync.dma_start(b_out[0], b_in[:])
                    nc.sync.dma_start(b_out[1], b_in[:])
                else:
                    nc.gpsimd.collective_compute(
                        "AllGather", ALU.bypass,
                        ins=[b_in.opt()], outs=[b_out.opt()],
                        replica_groups=[[0, 1], [2, 3], [4, 5], [6, 7]])
                pending[1] = b_out
                pending[0] = make_assembly(li)

            # ---------- head: pool -> conv1 -> conv2 -> fc1 -> fc2 ----------
            pending[0]()
            pending[0] = None
            # avg-pool(2): the 0.5 factor is folded into conv1 weights
            xp = act1.tile([P, 4, TH], BF16, tag="h2")
            nc.gpsimd.tensor_tensor(xp[:], x_sb[:, :, 0:T:2],
                                    x_sb[:, :, 1:T:2], ALU.add)
            # padded conv input [128, 4, 262] = [L(3) | own 256 | R(3)]
            xpe = act1.tile([P, 4, 262], BF16, tag="ysb")
            nc.gpsimd.tensor_copy(xpe[:, :, 3:259], xp[:, :, 0:TP])
            nc.gpsimd.tensor_scalar(xpe[:, :, 0:3], xp[:, :, 509:512],
                                    edges_sb[:, 0:1], None, ALU.mult)
            nc.gpsimd.tensor_scalar(xpe[:, :, 259:262], xp[:, :, 256:259],
                                    edges_sb[:, 1:2], None, ALU.mult)

            def conv_block(src, ntaps, w_dram, ncols, bn_s, bn_t, out):
                # w_dram: [ntaps, 4(mo), 128, 512] fp8 mo-blocked per tap
                for mo in range(4):
                    ps = ps_mm.tile([P, 512], F32, tag="ps")
                    for k in range(ntaps):
                        wt = wts.tile([P, 4, P], BF16, tag="cw")
                        nc.sync.dma_start(
                            wt[:].rearrange("p a b -> p (a b)"),
                            w_dram[k, mo])
                        for ko in range(4):
                            nc.tensor.matmul(
                                ps[:, 0:ncols], wt[:, ko],
                                src[:, ko, k:k + ncols],
                                start=(k == 0 and ko == 0),
                                stop=(k == ntaps - 1 and ko == 3))
                    nc.scalar.activation(out[:, mo, :], ps[:, 0:ncols],
                                         AF.Gelu, bias=bn_t[:, mo:mo + 1],
                                         scale=bn_s[:, mo:mo + 1])

            y1e = act1.tile([P, 4, 258], BF16, tag="r1")
            conv_block(xpe, 5, c1_wt, 258, bn1s_sb, bn1t_sb, y1e)
            # conv2 zero-pads its input at the GLOBAL sequence edges: kill the
            # computed y1 halo column on the outer side of each boundary core
            nc.gpsimd.tensor_scalar(y1e[:, :, 0:1], y1e[:, :, 0:1],
                                    edges_sb[:, 0:1], None, ALU.mult)
            nc.gpsimd.tensor_scalar(y1e[:, :, 257:258], y1e[:, :, 257:258],
                                    edges_sb[:, 1:2], None, ALU.mult)
            y2c = act1.tile([P, 4, TP], F32R, tag="h2")
            conv_block(y1e, 3, c2_wt, TP, bn2s_sb, bn2t_sb, y2c)
            # fc1 (512->256) + gelu  (fp32r for head accuracy)
            hfc = act1.tile([P, 2, TP], F32R, tag="qt")
            for mo in range(2):
                ps = ps_mm.tile([P, 512], F32, tag="ps")
                wt = wts.tile([P, 4 * P], F32R, tag="wmo")
                nc.sync.dma_start(wt[:], fc1_wt[mo])
                for ko in range(4):
                    nc.tensor.matmul(ps[:, 0:TP], wt[:, ko * P:(ko + 1) * P],
                                     y2c[:, ko, :],
                                     start=(ko == 0), stop=(ko == 3))
                nc.scalar.activation(hfc[:, mo, :], ps[:, 0:TP], AF.Gelu,
                                     bias=fc1b_sb[:, mo:mo + 1])
            # fc2 (256->1408 padded)
            ologit = act1.tile([P, 11, TP], F32, tag="r1")
            for mo in range(11):
                ps = ps_mm.tile([P, 512], F32, tag="ps")
                wt = wts.tile([P, 2 * P], F32R, tag="wmo")
                nc.sync.dma_start(wt[:], fc2_wt[mo])
                for ko in range(2):
                    nc.tensor.matmul(ps[:, 0:TP], wt[:, ko * P:(ko + 1) * P],
                                     hfc[:, ko, :],
                                     start=(ko == 0), stop=(ko == 1))
                nc.scalar.activation(ologit[:, mo, :], ps[:, 0:TP],
                                     AF.Identity, bias=fc2b_sb[:, mo:mo + 1])
            for mo in range(11):
                nc.sync.dma_start(out_d[mo * P:(mo + 1) * P, :],
                                  ologit[:, mo, :])

    nc.compile()
    _fix_act_tables(nc)
    return nc


def _fix_act_tables(nc):
    """Bass assigns each activation function its FIRST containing table from
    act_info.json (Ln -> natural_log, Exp -> exp_and_others), which makes
    alternating Ln/Exp reload tables every time (~4.5us each).  Rewrite: a
    table load whose following segment is servable by the still-loaded table
    is dropped; otherwise, if servable by natural_log_exp_and_others (which
    holds exp+ln+identity+relu+square), it is remapped there.  Walrus adopts
    pre-placed loads, and every activation's function remains inside the
    loaded set, so hardware semantics are unchanged."""
    from concourse.bacc import get_activation_tables
    tabs = list(get_activation_tables(nc.m.arch).items())
    target = next(i for i, (name, _) in enumerate(tabs)
                  if name == 'natural_log_exp_and_others')
    tfuncs = tabs[target][1]
    for blk in nc.m.functions[0].blocks:
        insts = blk.instructions
        # segment funcs following each load (until the next load)
        seg_funcs = {}
        cur_load = None
        for inst in insts:
            if isinstance(inst, mybir.InstLoadActFuncSet):
                cur_load = id(inst)
                seg_funcs[cur_load] = set()
            elif isinstance(inst, mybir.InstActivation) and cur_load is not None:
                seg_funcs[cur_load].add(inst.func)
        keep = []
        live = None                     # funcs of the currently loaded table
        for inst in insts:
            if isinstance(inst, mybir.InstLoadActFuncSet):
                seg = seg_funcs[id(inst)]
                if live is not None and seg <= live:
                    continue            # already servable: drop the load
                if seg <= tfuncs:
                    inst.act_func_set_id = target
                    live = tfuncs
                else:
                    live = tabs[inst.act_func_set_id][1]
            keep.append(inst)
        insts[:] = keep


def _q8(w):
    return np.clip(w, -448, 448).astype(ml_dtypes.float8_e4m3)


def _moblk8(w_t, nk, nm):
    """[nk*128, nm*128] fp32 -> [nm, 128, nk*128] fp8 DoubleRow pairs:
    out[mo, p, (kp,i,m)] = w_t[(2*kp+i)*128+p, mo*128+m] * SW"""
    a = _q8(w_t * SW).reshape(nk // 2, 2, P, nm, P)
    a = a.transpose(3, 2, 0, 1, 4)          # [nm, p, kp, i, m]
    return np.ascontiguousarray(a.reshape(nm, P, nk * P))


def _prep_inputs(inputs):
    """Host-side: transposes, padding, LN-affine folding, fp8 quantization,
    per-core shards."""
    f = lambda k: np.asarray(inputs[k], dtype=np.float32)
    poses = f('poses')
    embed_w, embed_b = f('embed_w'), f('embed_b')
    ln0_g, ln0_b = f('ln0_g'), f('ln0_b')
    inw, inb = f('inw'), f('inb')
    outw, outb = f('outw'), f('outb')
    ln1g, ln1b = f('ln1g'), f('ln1b')
    ln2g, ln2b = f('ln2g'), f('ln2b')
    ff1w, ff1b = f('ff1w'), f('ff1b')
    ff2w, ff2b = f('ff2w'), f('ff2b')
    conv1w, conv1b = f('conv1w'), f('conv1b')
    bn1g, bn1b, bn1m, bn1v = f('bn1g'), f('bn1b'), f('bn1m'), f('bn1v')
    conv2w, conv2b = f('conv2w'), f('conv2b')
    bn2g, bn2b, bn2m, bn2v = f('bn2g'), f('bn2b'), f('bn2m'), f('bn2v')
    fc1w, fc1b = f('fc1w'), f('fc1b')
    fc2w, fc2b = f('fc2w'), f('fc2b')

    def moblk(w_t, nk, nm):
        # fp32 [nk*128, nm*128] -> [nm, 128, nk*128] (plain, for head fcs)
        return np.ascontiguousarray(
            w_t.reshape(nk, P, nm, P).transpose(2, 1, 0, 3).reshape(nm, P, nk * P))

    shared = {}
    ewt = np.zeros((KIN, D), np.float32)
    ewt[:IN_DIM] = embed_w.T
    shared['emb_wt'] = moblk(ewt, 2, 4).astype(ml_dtypes.bfloat16)
    shared['emb_b'] = embed_b
    shared['ln0_g'] = -ln0_g

    qkv_wt = np.empty((NL, D, 3 * D), np.float32)
    qkv_bf = np.empty((NL, 3 * D), np.float32)
    out_wtf = np.empty((NL, D, D), np.float32)
    out_bf = np.empty((NL, D), np.float32)
    ff1_wtf = np.empty((NL, D, DFF), np.float32)
    ff1_bf = np.empty((NL, DFF), np.float32)
    ff2_wtf = np.empty((NL, DFF, D), np.float32)
    ff2_bf = np.empty((NL, D), np.float32)
    for l in range(NL):
        w = inw[l]                      # [3D, D]
        qkv_wt[l] = (w * ln1g[l][None, :]).T
        qkv_bf[l] = inb[l] + w @ ln1b[l]
        out_wtf[l] = outw[l].T
        bias_v = qkv_bf[l, 2 * D:]
        out_bf[l] = outb[l] + outw[l] @ bias_v
        ff1_wtf[l] = (ff1w[l] * ln2g[l][None, :]).T
        ff1_bf[l] = ff1b[l] + ff1w[l] @ ln2b[l]
        ff2_wtf[l] = ff2w[l].T
        ff2_bf[l] = ff2b[l]
    if 'fuse_flag' not in _CACHE:
        _CACHE['fuse_flag'] = bool(
            np.allclose(out_bf, 0.0) and np.allclose(ff2_bf, 0.0))
    qkv_bs = qkv_bf.copy()
    qkv_bs[:, :2 * D] *= SW             # q,k biases ride the x64 scale
    shared.update(qkv_b=qkv_bs, out_b=out_bf, ff1_b=ff1_bf,
                  ff2_b=ff2_bf)
    # LN outputs are emitted negated on-chip: fold the sign into every
    # weight that consumes an LN output (q,k,v,ff1) and into ln0_g
    shared['q_wt'] = np.stack([_moblk8(-qkv_wt[l][:, 0:D], 4, 4)
                               for l in range(NL)])
    shared['k_wt'] = np.stack([_moblk8(-qkv_wt[l][:, D:2 * D], 4, 4)
                               for l in range(NL)])
    # V weights in DoubleRow rhs layout [NL, p, kp, i, 512]
    vt = qkv_wt[:, :, 2 * D:].reshape(NL, 2, 2, P, D)   # [l, kp, i, p, v]
    shared['v_wt'] = np.ascontiguousarray(
        _q8(vt.transpose(0, 3, 1, 2, 4) * -SW))
    shared['out_wt'] = np.stack([_moblk8(out_wtf[l], 4, 4)
                                 for l in range(NL)])
    shared['ff1_wt'] = np.stack([moblk(-ff1_wtf[l], 4, 16)
                                 for l in range(NL)]).astype(ml_dtypes.bfloat16)
    shared['ff2_wt'] = np.stack([moblk(ff2_wtf[l], 16, 4)
                                 for l in range(NL)]).astype(ml_dtypes.bfloat16)

    bn1sc = bn1g / np.sqrt(bn1v + EPS)
    bn2sc = bn2g / np.sqrt(bn2v + EPS)
    c1t = conv1w.transpose(2, 1, 0) * 0.5           # [5, D_in, D_out]
    shared['c1_wt'] = np.stack([moblk(c1t[k], 4, 4)
                                for k in range(5)]).astype(ml_dtypes.bfloat16)
    shared['bn1_s'] = bn1sc
    shared['bn1_t'] = (conv1b - bn1m) * bn1sc + bn1b
    c2t = conv2w.transpose(2, 1, 0)
    shared['c2_wt'] = np.stack([moblk(c2t[k], 4, 4)
                                for k in range(3)]).astype(ml_dtypes.bfloat16)
    shared['bn2_s'] = bn2sc
    shared['bn2_t'] = (conv2b - bn2m) * bn2sc + bn2b
    shared['fc1_wt'] = moblk(np.ascontiguousarray(fc1w.T), 4, 2)
    shared['fc1_b'] = fc1b
    f2 = np.zeros((D // 2, NCP), np.float32)
    f2[:, :NCLS] = fc2w.T
    shared['fc2_wt'] = moblk(f2, 2, 11)
    f2b = np.zeros((NCP,), np.float32)
    f2b[:NCLS] = fc2b
    shared['fc2_b'] = f2b

    inv = 1.0 / (10000.0 ** (np.arange(0, D, 2, dtype=np.float32) / D))
    si = np.arange(T, dtype=np.float32)[:, None] * inv[None, :]
    pos = np.stack([np.sin(si), np.cos(si)], -1).reshape(T, D)
    pos = pos.astype(np.float32)
    pos_t_g = (pos + ln0_b[None, :]).T.copy()       # [D, T]

    in_maps = []
    for c in range(8):
        b, h = c // 2, c % 2
        own = slice(h * TH, (h + 1) * TH)
        peer = slice((1 - h) * TH, (2 - h) * TH)
        p_loc = np.concatenate([poses[b, own], poses[b, peer]], 0)
        pt = np.zeros((KIN, T), np.float32)
        pt[:IN_DIM] = p_loc.T
        pos_loc = np.concatenate([pos_t_g[:, own], pos_t_g[:, peer]], 1)
        edges_a = np.zeros((P, 2), np.float32)
        edges_a[:, 0] = 1.0 if h == 1 else 0.0
        edges_a[:, 1] = 1.0 if h == 0 else 0.0
        m = dict(shared)
        m['poses_t'] = pt.astype(ml_dtypes.bfloat16)
        m['pos_t'] = pos_loc
        m['edges'] = edges_a
        in_maps.append({k: np.ascontiguousarray(v) for k, v in m.items()})
    return in_maps


def _get_runner():
    """Build the module once and cache a jitted SPMD executable whose weight
    operands stay device-resident between calls."""
    if 'runner' in _CACHE:
        return _CACHE['runner']
    fuse = _CACHE.get('fuse_flag', True)
    import jax
    import concourse.mybir as mybir_
    from concourse import bass2jax
    from jax.experimental.shard_map import shard_map
    from jax.sharding import Mesh, NamedSharding, PartitionSpec

    nc = _build(fuse_bias=fuse)
    bass2jax.install_neuronx_cc_hook()
    partition_name = (nc.partition_id_tensor.name
                      if nc.partition_id_tensor else None)
    in_names, out_names, out_avals, zero_outs = [], [], [], []
    for alloc in nc.m.functions[0].allocations:
        if not isinstance(alloc, mybir_.MemoryLocationSet):
            continue
        name = alloc.memorylocations[0].name
        if alloc.kind == "ExternalInput":
            if name != partition_name:
                in_names.append(name)
        elif alloc.kind == "ExternalOutput":
            shape = tuple(alloc.tensor_shape)
            dtype = mybir_.dt.np(alloc.dtype)
            out_names.append(name)
            out_avals.append(jax.core.ShapedArray(shape, dtype))
            zero_outs.append((shape, dtype))
    n_params = len(in_names)
    all_names = in_names + out_names
    if partition_name is not None:
        all_names.append(partition_name)
    donate = tuple(range(n_params, n_params + len(out_names)))

    def _body(*args):
        operands = list(args)
        if partition_name is not None:
            operands.append(bass2jax.partition_id_tensor())
        outs = bass2jax._bass_exec_p.bind(
            *operands,
            out_avals=tuple(out_avals),
            in_names=tuple(all_names),
            out_names=tuple(out_names),
            lowering_input_output_aliases=(),
            sim_require_finite=True,
            sim_require_nnan=True,
            nc=nc,
        )
        return tuple(outs)

    devices = jax.devices()[:8]
    mesh = Mesh(np.asarray(devices), ("core",))
    spec = PartitionSpec("core")
    sharding = NamedSharding(mesh, spec)
    jitted = jax.jit(
        shard_map(_body, mesh=mesh, in_specs=(spec,) * (n_params + len(out_names)),
                  out_specs=(spec,) * len(out_names), check_rep=False),
        donate_argnums=donate, keep_unused=True)

    runner = dict(jitted=jitted, in_names=in_names, out_names=out_names,
                  zero_outs=zero_outs, sharding=sharding)
    _CACHE['runner'] = runner
    return runner


def _put_args(in_maps):
    import jax
    r = _get_runner()
    args = []
    for name in r['in_names']:
        concat = np.concatenate([in_maps[c][name] for c in range(8)], axis=0)
        args.append(jax.device_put(concat, r['sharding']))
    return args


def _exec(args):
    """Run with device-resident input args; returns per-core result dicts.
    Output (donated) buffers are freshly zero-allocated per call."""
    import jax
    r = _get_runner()
    outs_in = [jax.device_put(np.zeros((8 * s[0],) + s[1:], d), r['sharding'])
               for s, d in r['zero_outs']]
    outs = r['jitted'](*args, *outs_in)
    outs = [np.asarray(o) for o in outs]
    return [{name: outs[i].reshape(8, *r['zero_outs'][i][0])[c]
             for i, name in enumerate(r['out_names'])}
            for c in range(8)]


def _run(in_maps):
    return _exec(_put_args(in_maps))


def kernel(**inputs):
    in_maps = _prep_inputs(inputs)
    results = _run(in_maps)
    out = np.empty((B, T // 2, NCLS), np.float32)
    for c in range(8):
        b, h = c // 2, c % 2
        out[b, h * TP:(h + 1) * TP, :] = results[c]['out'][:NCLS].T
    return out
